# revision 1
# baseline (speedup 1.0000x reference)
"""Two-level VQ assignment kernel for Trainium2 (8 NeuronCores, data-parallel).

Per core: stream 125k samples; stage-1 outer-cluster scores are computed
born-transposed ([sample, cluster]) via x-as-weights matmuls with bf16
hi/lo split for fp32-class accuracy; DVE max/max_index gives the outer
argmin; an indirect DMA gathers the winning cluster's inner-center row
(with norms) per sample; a broadcast multiply + segmented reduce + DVE
max/max_index gives the inner argmin.
"""

import sys

sys.path.insert(0, "/opt/trn_rl_repo")

import numpy as np
import ml_dtypes

import concourse.bass as bass
import concourse.bacc as bacc
import concourse.mybir as mybir
from concourse import tile
from concourse.bass_types import AP
from concourse.bass_utils import run_bass_kernel_spmd
import jax
from jax.sharding import Mesh, PartitionSpec
from jax.experimental.shard_map import shard_map

N = 1_000_000
D1, D2 = 16, 16
NC, NCPC = 64, 8
NCORES = 8
NPC = N // NCORES          # samples per core
P = 128                    # partition tile
NT = (NPC + P - 1) // P    # 977 sample-tiles per core
NP = NT * P                # padded per-core sample count (125056)
TB = 16                    # tiles staged per output DMA
BF16 = mybir.dt.bfloat16
F32 = mybir.dt.float32
U32 = mybir.dt.uint32

_cache = {}
TRACE = False
STAGE = 5   # 1=dma+mm, 2=+copy, 3=+s1argmax, 4=+gather+prods+reduce, 5=full


def _build_kernel():
    nc = bacc.Bacc()
    xh = nc.dram_tensor("xh", [33, NP], BF16, kind="ExternalInput")
    xl = nc.dram_tensor("xl", [33, NP], BF16, kind="ExternalInput")
    x2 = nc.dram_tensor("x2", [NP, 17], F32, kind="ExternalInput")
    tab = nc.dram_tensor("tab", [NC, 136], F32, kind="ExternalInput")
    w4 = nc.dram_tensor("w4", [33, 4 * NC], BF16, kind="ExternalInput")
    out = nc.dram_tensor("out", [P, NT], U32, kind="ExternalOutput")

    with tile.TileContext(nc) as tc:
        with (
            tc.tile_pool(name="const", bufs=1) as cpool,
            tc.tile_pool(name="xin", bufs=6) as xpool,
            tc.tile_pool(name="s1pool", bufs=3, space="PSUM") as s1pool,
            tc.tile_pool(name="work", bufs=4) as wpool,
            tc.tile_pool(name="opool", bufs=3) as opool,
        ):
            w = cpool.tile([33, 4 * NC], BF16)
            nc.sync.dma_start(w[:], w4[:])
            wv = [w[:, i * NC:(i + 1) * NC] for i in range(4)]

            for tb in range(NT // TB + (1 if NT % TB else 0)):
                t0 = tb * TB
                t1 = min(t0 + TB, NT)
                obuf = opool.tile([P, TB], U32)
                for t in range(t0, t1):
                    sl = slice(t * P, (t + 1) * P)
                    xh_t = xpool.tile([33, P], BF16, tag="xh")
                    xl_t = xpool.tile([33, P], BF16, tag="xl")
                    x2_t = xpool.tile([P, 17], F32, tag="x2")
                    nc.sync.dma_start(xh_t[:], xh[:, sl])
                    nc.sync.dma_start(xl_t[:], xl[:, sl])
                    nc.sync.dma_start(x2_t[:], x2[sl, :])

                    s1p = s1pool.tile([P, NC], F32, tag="s1p")
                    nc.tensor.matmul(s1p[:], xh_t[:], wv[0], start=True, stop=False)
                    nc.tensor.matmul(s1p[:], xh_t[:], wv[1], start=False, stop=False)
                    nc.tensor.matmul(s1p[:], xl_t[:], wv[2], start=False, stop=False)
                    nc.tensor.matmul(s1p[:], xl_t[:], wv[3], start=False, stop=True)

                    if STAGE < 2:
                        continue
                    s1 = wpool.tile([P, NC], F32, tag="s1")
                    nc.scalar.activation(s1[:], s1p[:], mybir.ActivationFunctionType.Copy)

                    if STAGE < 3:
                        continue
                    mx8 = wpool.tile([P, 8], F32, tag="mx8")
                    mi8 = wpool.tile([P, 8], U32, tag="mi8")
                    nc.vector.max(mx8[:], s1[:])
                    nc.vector.max_index(mi8[:], mx8[:], s1[:])

                    if STAGE < 4:
                        continue
                    cc = wpool.tile([P, 136], F32, tag="cc")
                    nc.gpsimd.indirect_dma_start(
                        out=cc[:],
                        out_offset=None,
                        in_=tab[:],
                        in_offset=bass.IndirectOffsetOnAxis(ap=mi8[:, 0:1], axis=0),
                    )

                    prods = wpool.tile([P, 136], F32, tag="prods")
                    x2b = AP(x2_t.tensor, x2_t[:].offset, [[17, P], [0, 8], [1, 17]])
                    nc.gpsimd.tensor_tensor(
                        out=prods[:].rearrange("p (k f) -> p k f", k=8),
                        in0=x2b,
                        in1=cc[:].rearrange("p (k f) -> p k f", k=8),
                        op=mybir.AluOpType.mult,
                    )
                    u2 = wpool.tile([P, 8], F32, tag="u2")
                    nc.vector.tensor_reduce(
                        u2[:],
                        prods[:].rearrange("p (k f) -> p k f", k=8),
                        axis=mybir.AxisListType.X,
                        op=mybir.AluOpType.add,
                    )
                    if STAGE < 5:
                        continue
                    k8v = wpool.tile([P, 8], F32, tag="k8v")
                    k8 = wpool.tile([P, 8], U32, tag="k8")
                    nc.vector.max(k8v[:], u2[:])
                    nc.vector.max_index(k8[:], k8v[:], u2[:])

                    jid = wpool.tile([P, 1], U32, tag="jid")
                    nc.gpsimd.tensor_scalar(
                        out=jid[:], in0=mi8[:, 0:1], scalar1=8, scalar2=None,
                        op0=mybir.AluOpType.mult,
                    )
                    nc.gpsimd.tensor_tensor(
                        out=obuf[:, t - t0:t - t0 + 1], in0=jid[:], in1=k8[:, 0:1],
                        op=mybir.AluOpType.add,
                    )
                nc.gpsimd.dma_start(out[:, t0:t1], obuf[:, :t1 - t0])
    nc.compile()
    return nc


def _prep_inputs(x, centers_outer, centers_inner):
    x = np.asarray(x, dtype=np.float32)
    co = np.asarray(centers_outer, dtype=np.float32)
    ci = np.asarray(centers_inner, dtype=np.float32)

    # stage-1 weight tables: u_j = 2*x1.c_j - |c_j|^2 (argmax == argmin dist)
    c2 = 2.0 * co.T                                   # [16, 64]
    ch = c2.astype(ml_dtypes.bfloat16)
    cl = (c2 - ch.astype(np.float32)).astype(ml_dtypes.bfloat16)
    bias = -np.sum(co * co, axis=1)                   # [64]
    bh = bias.astype(ml_dtypes.bfloat16)
    bl = (bias - bh.astype(np.float32)).astype(ml_dtypes.bfloat16)
    w4 = np.zeros((33, 4 * NC), dtype=ml_dtypes.bfloat16)
    w4[0:16, 0 * NC:1 * NC] = ch
    w4[32, 0 * NC:1 * NC] = bh
    w4[0:16, 1 * NC:2 * NC] = cl
    w4[32, 1 * NC:2 * NC] = bl
    w4[0:16, 2 * NC:3 * NC] = ch
    w4[0:16, 3 * NC:4 * NC] = cl

    # stage-2 gather table: row j = concat_k [2*cc_jk (16), -|cc_jk|^2]
    tabv = np.zeros((NC, 136), dtype=np.float32)
    t = tabv.reshape(NC, 8, 17)
    t[:, :, 0:16] = 2.0 * ci
    t[:, :, 16] = -np.sum(ci * ci, axis=2)

    xt = np.ascontiguousarray(x.T)                    # [32, N]
    xth = xt.astype(ml_dtypes.bfloat16)
    xtl = (xt - xth.astype(np.float32)).astype(ml_dtypes.bfloat16)

    in_maps = []
    for c in range(NCORES):
        sl = slice(c * NPC, (c + 1) * NPC)
        xhc = np.zeros((33, NP), dtype=ml_dtypes.bfloat16)
        xlc = np.zeros((33, NP), dtype=ml_dtypes.bfloat16)
        xhc[0:32, :NPC] = xth[:, sl]
        xhc[32, :NPC] = np.float32(1.0)
        xlc[0:32, :NPC] = xtl[:, sl]
        x2c = np.zeros((NP, 17), dtype=np.float32)
        x2c[:NPC, 0:16] = x[sl, 16:32]
        x2c[:NPC, 16] = 1.0
        in_maps.append({
            "xh": xhc, "xl": xlc, "x2": x2c,
            "tab": tabv, "w4": w4,
        })
    return in_maps


def _get_exec(nc):
    # mirror bass2jax.run_bass_via_pjrt but keep the jitted callable + device
    # inputs so repeated executions can be timed without re-transfer
    from concourse import bass2jax
    import concourse.mybir as _mb
    bass2jax.install_neuronx_cc_hook()
    partition_name = nc.partition_id_tensor.name if nc.partition_id_tensor else None
    in_names, out_names, out_avals, zero_outs = [], [], [], []
    for alloc in nc.m.functions[0].allocations:
        if not isinstance(alloc, _mb.MemoryLocationSet):
            continue
        name = alloc.memorylocations[0].name
        if alloc.kind == "ExternalInput":
            if name != partition_name:
                in_names.append(name)
        elif alloc.kind == "ExternalOutput":
            out_names.append(name)
            out_avals.append(jax.core.ShapedArray(
                tuple(alloc.tensor_shape), _mb.dt.np(alloc.dtype)))
            zero_outs.append(np.zeros(tuple(alloc.tensor_shape),
                                      _mb.dt.np(alloc.dtype)))
    n_params = len(in_names)
    in_names = in_names + out_names
    if partition_name is not None:
        in_names.append(partition_name)

    def _body(*args):
        operands = list(args)
        if partition_name is not None:
            operands.append(bass2jax.partition_id_tensor())
        outs = bass2jax._bass_exec_p.bind(
            *operands,
            out_avals=tuple(out_avals),
            in_names=tuple(in_names),
            out_names=tuple(out_names),
            lowering_input_output_aliases=(),
            sim_require_finite=True,
            sim_require_nnan=True,
            nc=nc,
        )
        return tuple(outs)

    devices = jax.devices()[:NCORES]
    mesh = Mesh(np.asarray(devices), ("core",))
    n_outs = len(out_names)
    sharded = jax.jit(
        shard_map(_body, mesh=mesh,
                  in_specs=(PartitionSpec("core"),) * (n_params + n_outs),
                  out_specs=(PartitionSpec("core"),) * n_outs,
                  check_rep=False),
        keep_unused=True,
    )
    return sharded, in_names[:n_params], out_names, out_avals, zero_outs, mesh


def _execute(in_maps, time_iters=0):
    if "nc" not in _cache:
        _cache["nc"] = _build_kernel()
        _cache["exec"] = _get_exec(_cache["nc"])
    sharded, in_names, out_names, out_avals, zero_outs, mesh = _cache["exec"]
    concat_in = [
        np.concatenate([m[name] for m in in_maps], axis=0) for name in in_names
    ]
    concat_zeros = [
        np.zeros((NCORES * z.shape[0], *z.shape[1:]), z.dtype) for z in zero_outs
    ]
    import time as _time
    din = [jax.device_put(a) for a in concat_in]
    dzero = [jax.device_put(z) for z in concat_zeros]
    out_arrs = sharded(*din, *dzero)
    jax.block_until_ready(out_arrs)
    if time_iters:
        times = []
        for _ in range(time_iters):
            t0 = _time.perf_counter()
            out_arrs = sharded(*din, *dzero)
            jax.block_until_ready(out_arrs)
            times.append(_time.perf_counter() - t0)
        _cache["exec_ns"] = int(min(times) * 1e9)
    return [
        {name: np.asarray(out_arrs[i]).reshape(NCORES, *out_avals[i].shape)[c]
         for i, name in enumerate(out_names)}
        for c in range(NCORES)
    ]


def kernel(x, centers_outer, centers_inner):
    in_maps = _prep_inputs(x, centers_outer, centers_inner)
    results = _execute(in_maps, time_iters=3 if TRACE else 0)
    outs = []
    for c in range(NCORES):
        o = results[c]["out"]           # [128, NT] uint32, sample (t,p) at [p,t]
        outs.append(o.T.reshape(-1)[:NPC])
    return np.concatenate(outs).astype(np.int32)

